# revision 11
# baseline (speedup 1.0000x reference)
"""Trainium2 Bass kernel for nn_DecoderSeqAdvanced (ragged decoder sequence).

The ragged gather idx selects, for each batch b, rows t in [0, seq_len[b]-1)
-- a prefix of each sequence. Every output row is an independent function of
one input row, so the host flattens all N ragged rows, splits them
contiguously into 8 equal chunks (one per NeuronCore), and ships each chunk
pre-transposed as hidT [256, ncmax]. A single dense SPMD program per core
computes, per 512-row group:

    projT [128,512] = fc_wT.T @ hT + fc_b      (PE, K=256, f32r)
    raw   [128,512]x4 = projT.T @ meta_data.T  (PE, K=128, f32r)
    h1    [64,512]  = relu(l1_w @ [div;yrs;proj] + l1_b)
    z     [128,1]x4 = h1.T @ l2_wT             (rows on partitions)

z columns accumulate into [128, ntiles]; sigmoid/mix/log run once at the end
across all partitions. Only DRAM traffic is hidT in and raw/logw out -- the
memory-regime floor (~3KB per ragged row).
"""

import numpy as np

import concourse.bass as bass
import concourse.tile as tile
from concourse import bacc, mybir
from concourse import bass_utils

B, T, H, D, V, PH = 64, 2048, 256, 128, 512, 64
NCORES = 8
P = 128          # partition dim
G = 512          # rows per group (four 128-row chunks)
F32 = mybir.dt.float32
F32R = mybir.dt.float32r  # fast PE mode, 4x fp32 throughput at free>=256
BF16 = mybir.dt.bfloat16

_CACHE: dict = {}

_MODES = {"bf16": BF16, "f32r": F32R, "f32": F32}


def _build(ncmax: int, mode: str):
    """Build + compile the single-core Bass program for ncmax packed rows."""
    nc = bacc.Bacc("TRN2", target_bir_lowering=False, debug=False,
                   num_devices=NCORES)
    MMDT = _MODES[mode]
    # f32r is ISA-restricted to wide moving operands; small matmuls (z, K=2)
    # use plain f32 in that mode. bf16 is legal everywhere.
    SMDT = BF16 if mode == "bf16" else F32
    ntiles = ncmax // P
    ngroups = ncmax // G

    hidT = nc.dram_tensor("hidT", [H, ncmax], MMDT, kind="ExternalInput").ap()
    dy = nc.dram_tensor("dy", [2, ncmax], SMDT, kind="ExternalInput").ap()
    gpc = nc.dram_tensor("gpc", [P, ntiles], F32, kind="ExternalInput").ap()
    fcwT = nc.dram_tensor("fcwT", [H, D], MMDT, kind="ExternalInput").ap()
    fcb = nc.dram_tensor("fcb", [D, 1], F32, kind="ExternalInput").ap()
    metaT = nc.dram_tensor("metaT", [D, V], MMDT, kind="ExternalInput").ap()
    l1wT = nc.dram_tensor("l1wT", [D, PH], MMDT, kind="ExternalInput").ap()
    l1w01T = nc.dram_tensor("l1w01T", [2, PH], SMDT, kind="ExternalInput").ap()
    l1b = nc.dram_tensor("l1b", [PH, 1], F32, kind="ExternalInput").ap()
    l2wT = nc.dram_tensor("l2wT", [PH, 1], SMDT, kind="ExternalInput").ap()
    l2b = nc.dram_tensor("l2b", [P, 1], F32, kind="ExternalInput").ap()
    raw = nc.dram_tensor("raw", [ncmax, V], F32, kind="ExternalOutput").ap()
    logwc = nc.dram_tensor("logwc", [P, ntiles], F32,
                           kind="ExternalOutput").ap()

    hidT_r = hidT.rearrange("(a p) r -> p a r", p=P)

    with tile.TileContext(nc) as tc:
        with (
            tc.tile_pool(name="consts", bufs=1) as consts,
            tc.tile_pool(name="projcache", bufs=ngroups) as projcache,
            tc.tile_pool(name="io3", bufs=6) as io3,
            tc.tile_pool(name="work", bufs=3) as work,
            tc.tile_pool(name="psA", bufs=3, space="PSUM") as psA,
            tc.tile_pool(name="psR", bufs=3, space="PSUM") as psR,
            tc.tile_pool(name="psB", bufs=1, space="PSUM") as psB,
            tc.tile_pool(name="psC", bufs=1, space="PSUM") as psC,
        ):
            fcwT_sb = consts.tile([P, 2, D], MMDT)
            nc.scalar.dma_start(fcwT_sb, fcwT.rearrange("(a p) d -> p a d", p=P))
            metaT_sb = consts.tile([D, V], MMDT)
            nc.gpsimd.dma_start(metaT_sb, metaT)
            l1wT_sb = consts.tile([D, PH], MMDT)
            nc.scalar.dma_start(l1wT_sb, l1wT)
            l1w01T_sb = consts.tile([2, PH], SMDT)
            nc.scalar.dma_start(l1w01T_sb, l1w01T)
            l2wT_sb = consts.tile([PH, 1], SMDT)
            nc.scalar.dma_start(l2wT_sb, l2wT)
            fcb_sb = consts.tile([D, 1], F32)
            nc.gpsimd.dma_start(fcb_sb, fcb)
            l1b_sb = consts.tile([PH, 1], F32)
            nc.gpsimd.dma_start(l1b_sb, l1b)
            l2b_sb = consts.tile([P, 1], F32)
            nc.gpsimd.dma_start(l2b_sb, l2b)
            dy_sb = consts.tile([2, ncmax], SMDT)
            nc.scalar.dma_start(dy_sb, dy)
            gp_sb = consts.tile([P, ntiles], F32)
            nc.gpsimd.dma_start(gp_sb, gpc)
            zall = consts.tile([P, ntiles], F32)
            projs = []

            # phase 1: dense proj matmuls; projT for the whole core
            # stays cached in SBUF (1KB/partition per group)
            for g in range(ngroups):
                r0 = g * G
                hT = io3.tile([P, 2, G], MMDT, tag="hT")
                nc.scalar.dma_start(hT, hidT_r[:, :, r0:r0 + G])

                projT_ps = psA.tile([P, G], F32, tag="projT")
                nc.tensor.matmul(projT_ps, fcwT_sb[:, 0, :], hT[:, 0, :],
                                 start=True, stop=False)
                nc.tensor.matmul(projT_ps, fcwT_sb[:, 1, :], hT[:, 1, :],
                                 start=False, stop=True)
                projT_sb = projcache.tile([P, G], MMDT, tag="projT_sb")
                projs.append(projT_sb)
                for q in range(4):
                    sl = slice(q * P, (q + 1) * P)
                    if q % 2 == 0:
                        nc.vector.tensor_scalar_add(projT_sb[:, sl],
                                                    projT_ps[:, sl], fcb_sb)
                    else:
                        nc.scalar.activation(
                            projT_sb[:, sl], projT_ps[:, sl],
                            mybir.ActivationFunctionType.Identity,
                            bias=fcb_sb)

            # phase 2: raw matmuls + MLP, one output DMA per group
            for g in range(ngroups):
                r0 = g * G
                projT_sb = projs[g]
                raw_sb = io3.tile([P, 4, V], F32, tag="rawsb")
                for c in range(4):
                    raw_ps = psR.tile([P, V], F32, tag="raw")
                    nc.tensor.matmul(raw_ps, projT_sb[:, c * P:(c + 1) * P],
                                     metaT_sb, start=True, stop=True)
                    if c == 3:
                        nc.scalar.copy(raw_sb[:, c, :], raw_ps)
                    else:
                        nc.vector.tensor_copy(raw_sb[:, c, :], raw_ps)
                eng = nc.sync if g % 2 == 0 else nc.gpsimd
                eng.dma_start(
                    raw[r0:r0 + G, :].rearrange("(c p) v -> p c v", p=P),
                    raw_sb)

                h1_ps = psB.tile([PH, G], F32, tag="h1")
                nc.tensor.matmul(h1_ps, l1w01T_sb, dy_sb[:, r0:r0 + G],
                                 start=True, stop=False)
                nc.tensor.matmul(h1_ps, l1wT_sb, projT_sb,
                                 start=False, stop=True)
                h1_sb = work.tile([PH, G], SMDT, tag="h1sb")
                nc.scalar.activation(h1_sb, h1_ps,
                                     mybir.ActivationFunctionType.Relu,
                                     bias=l1b_sb)

                z_ps = psC.tile([P, 4], F32, tag="z")
                for c in range(4):
                    nc.tensor.matmul(z_ps[:, c:c + 1],
                                     h1_sb[:, c * P:(c + 1) * P], l2wT_sb,
                                     start=True, stop=True)
                nc.vector.tensor_copy(zall[:, g * 4:(g + 1) * 4], z_ps)

            # tail: ps = sigmoid(z + l2_b); w = ps*(1-2g) + g; logw = ln(w)
            onem2g = consts.tile([P, ntiles], F32)
            nc.vector.tensor_scalar(onem2g, gp_sb, -2.0, 1.0,
                                    mybir.AluOpType.mult, mybir.AluOpType.add)
            ps_all = consts.tile([P, ntiles], F32)
            nc.scalar.activation(ps_all, zall,
                                 mybir.ActivationFunctionType.Sigmoid,
                                 bias=l2b_sb)
            w_all = consts.tile([P, ntiles], F32)
            nc.vector.tensor_mul(w_all, ps_all, onem2g)
            nc.vector.tensor_add(w_all, w_all, gp_sb)
            logw_sb = consts.tile([P, ntiles], F32)
            nc.scalar.activation(logw_sb, w_all,
                                 mybir.ActivationFunctionType.Ln)
            nc.sync.dma_start(logwc, logw_sb)

    nc.compile()
    return nc


def _get_program(ncmax: int, mode: str):
    key = (ncmax, mode)
    if key not in _CACHE:
        _CACHE[key] = _build(ncmax, mode)
    return _CACHE[key]


def kernel(hidden_seq, seq_len, gp_sim_mask, div, yrs, meta_data,
           fc_w, fc_b, l1_w, l1_b, l2_w, l2_b, mode="bf16", trace=False):
    hidden_seq = np.asarray(hidden_seq, np.float32)
    sl = np.asarray(seq_len).astype(np.int64)
    gp = np.asarray(gp_sim_mask, np.float32).reshape(B * T)
    dv = np.asarray(div, np.float32).reshape(B * T)
    yr = np.asarray(yrs, np.float32).reshape(B * T)
    meta_data = np.asarray(meta_data, np.float32)
    fc_w = np.asarray(fc_w, np.float32)
    fc_b = np.asarray(fc_b, np.float32)
    l1_w = np.asarray(l1_w, np.float32)
    l1_b = np.asarray(l1_b, np.float32)
    l2_w = np.asarray(l2_w, np.float32)
    l2_b = np.asarray(l2_b, np.float32)

    # ragged row indices: for batch b, rows b*T + [0, seq_len[b]-1)
    lens = sl - 1
    N = int(lens.sum())
    idx = np.empty(N, np.int64)
    pos = 0
    for b in range(B):
        l = int(lens[b])
        idx[pos:pos + l] = b * T + np.arange(l)
        pos += l

    # contiguous split across cores
    bounds = [(N * c) // NCORES for c in range(NCORES + 1)]
    chunk = max(bounds[c + 1] - bounds[c] for c in range(NCORES))
    ncmax = max(G, ((chunk + G - 1) // G) * G)
    ntiles = ncmax // P

    import ml_dtypes
    mmnp = ml_dtypes.bfloat16 if mode == "bf16" else np.float32
    smnp = ml_dtypes.bfloat16 if mode == "bf16" else np.float32
    hid_flat = hidden_seq.reshape(B * T, H)
    shared = {
        "fcwT": np.ascontiguousarray(fc_w.T).astype(mmnp),
        "fcb": np.ascontiguousarray(fc_b.reshape(D, 1)),
        "metaT": np.ascontiguousarray(meta_data.T).astype(mmnp),
        "l1wT": np.ascontiguousarray(l1_w[:, 2:].T).astype(mmnp),
        "l1w01T": np.ascontiguousarray(l1_w[:, :2].T).astype(smnp),
        "l1b": np.ascontiguousarray(l1_b.reshape(PH, 1)),
        "l2wT": np.ascontiguousarray(l2_w.T).astype(smnp),
        "l2b": np.full((P, 1), np.float32(l2_b.reshape(1)[0])),
    }
    in_maps = []
    for c in range(NCORES):
        s, e = bounds[c], bounds[c + 1]
        n_c = e - s
        hidT_c = np.zeros((H, ncmax), mmnp)
        hidT_c[:, :n_c] = hid_flat[idx[s:e]].T.astype(mmnp)
        dy_c = np.zeros((2, ncmax), smnp)
        dy_c[0, :n_c] = dv[idx[s:e]].astype(smnp)
        dy_c[1, :n_c] = yr[idx[s:e]].astype(smnp)
        gp_c = np.zeros(ncmax, np.float32)
        gp_c[:n_c] = gp[idx[s:e]]
        gpc_c = np.ascontiguousarray(gp_c.reshape(ntiles, P).T)
        in_maps.append({"hidT": hidT_c, "dy": dy_c, "gpc": gpc_c, **shared})

    nc = _get_program(ncmax, mode)
    res = bass_utils.run_bass_kernel_spmd(nc, in_maps,
                                          core_ids=list(range(NCORES)),
                                          trace=trace)

    raw_out = np.empty((N, V), np.float32)
    logw_out = np.empty((N, 1), np.float32)
    for c in range(NCORES):
        s, e = bounds[c], bounds[c + 1]
        raw_out[s:e] = res.results[c]["raw"][:e - s]
        logw_flat = res.results[c]["logwc"].T.reshape(-1)
        logw_out[s:e, 0] = logw_flat[:e - s]
    if trace:
        kernel.last_results = res
    return raw_out, logw_out


# revision 12
# speedup vs baseline: 1.2003x; 1.2003x over previous
"""Trainium2 Bass kernel for nn_DecoderSeqAdvanced (ragged decoder sequence).

The ragged gather idx selects, for each batch b, rows t in [0, seq_len[b]-1)
-- a prefix of each sequence. Every output row is an independent function of
one input row, so the host flattens all N ragged rows, splits them
contiguously into 8 equal chunks (one per NeuronCore), and ships each chunk
pre-transposed as hidT [256, ncmax]. A single dense SPMD program per core
computes, per 512-row group:

    projT [128,512] = fc_wT.T @ hT + fc_b      (PE, K=256, f32r)
    raw   [128,512]x4 = projT.T @ meta_data.T  (PE, K=128, f32r)
    h1    [64,512]  = relu(l1_w @ [div;yrs;proj] + l1_b)
    z     [128,1]x4 = h1.T @ l2_wT             (rows on partitions)

z columns accumulate into [128, ntiles]; sigmoid/mix/log run once at the end
across all partitions. Only DRAM traffic is hidT in and raw/logw out -- the
memory-regime floor (~3KB per ragged row).
"""

import numpy as np

import concourse.bass as bass
import concourse.tile as tile
from concourse import bacc, mybir
from concourse import bass_utils

B, T, H, D, V, PH = 64, 2048, 256, 128, 512, 64
NCORES = 8
P = 128          # partition dim
G = 512          # rows per group (four 128-row chunks)
F32 = mybir.dt.float32
F32R = mybir.dt.float32r  # fast PE mode, 4x fp32 throughput at free>=256
BF16 = mybir.dt.bfloat16

_CACHE: dict = {}

_MODES = {"bf16": BF16, "f32r": F32R, "f32": F32}


def _build(ncmax: int, mode: str):
    """Build + compile the single-core Bass program for ncmax packed rows."""
    nc = bacc.Bacc("TRN2", target_bir_lowering=False, debug=False,
                   num_devices=NCORES)
    MMDT = _MODES[mode]
    # f32r is ISA-restricted to wide moving operands; small matmuls (z, K=2)
    # use plain f32 in that mode. bf16 is legal everywhere.
    SMDT = BF16 if mode == "bf16" else F32
    ntiles = ncmax // P
    ngroups = ncmax // G

    hidT = nc.dram_tensor("hidT", [H, ncmax], MMDT, kind="ExternalInput").ap()
    dy = nc.dram_tensor("dy", [2, ncmax], SMDT, kind="ExternalInput").ap()
    gpc = nc.dram_tensor("gpc", [P, ntiles], F32, kind="ExternalInput").ap()
    fcwT = nc.dram_tensor("fcwT", [H, D], MMDT, kind="ExternalInput").ap()
    fcb = nc.dram_tensor("fcb", [D, 1], F32, kind="ExternalInput").ap()
    metaT = nc.dram_tensor("metaT", [D, V], MMDT, kind="ExternalInput").ap()
    l1wT = nc.dram_tensor("l1wT", [D, PH], MMDT, kind="ExternalInput").ap()
    l1w01T = nc.dram_tensor("l1w01T", [2, PH], SMDT, kind="ExternalInput").ap()
    l1b = nc.dram_tensor("l1b", [PH, 1], F32, kind="ExternalInput").ap()
    l2wT = nc.dram_tensor("l2wT", [PH, 1], SMDT, kind="ExternalInput").ap()
    l2b = nc.dram_tensor("l2b", [P, 1], F32, kind="ExternalInput").ap()
    raw = nc.dram_tensor("raw", [ncmax, V], F32, kind="ExternalOutput").ap()
    logwc = nc.dram_tensor("logwc", [P, ntiles], F32,
                           kind="ExternalOutput").ap()

    hidT_r = hidT.rearrange("(a p) r -> p a r", p=P)

    with tile.TileContext(nc) as tc:
        with (
            tc.tile_pool(name="consts", bufs=1) as consts,
            tc.tile_pool(name="io3", bufs=3) as io3,
            tc.tile_pool(name="work", bufs=3) as work,
            tc.tile_pool(name="psA", bufs=3, space="PSUM") as psA,
            tc.tile_pool(name="psR", bufs=3, space="PSUM") as psR,
            tc.tile_pool(name="psB", bufs=1, space="PSUM") as psB,
            tc.tile_pool(name="psC", bufs=1, space="PSUM") as psC,
        ):
            fcwT_sb = consts.tile([P, 2, D], MMDT)
            nc.scalar.dma_start(fcwT_sb, fcwT.rearrange("(a p) d -> p a d", p=P))
            metaT_sb = consts.tile([D, V], MMDT)
            nc.gpsimd.dma_start(metaT_sb, metaT)
            l1wT_sb = consts.tile([D, PH], MMDT)
            nc.scalar.dma_start(l1wT_sb, l1wT)
            l1w01T_sb = consts.tile([2, PH], SMDT)
            nc.scalar.dma_start(l1w01T_sb, l1w01T)
            l2wT_sb = consts.tile([PH, 1], SMDT)
            nc.scalar.dma_start(l2wT_sb, l2wT)
            fcb_sb = consts.tile([D, 1], F32)
            nc.gpsimd.dma_start(fcb_sb, fcb)
            l1b_sb = consts.tile([PH, 1], F32)
            nc.gpsimd.dma_start(l1b_sb, l1b)
            l2b_sb = consts.tile([P, 1], F32)
            nc.gpsimd.dma_start(l2b_sb, l2b)
            dy_sb = consts.tile([2, ncmax], SMDT)
            nc.scalar.dma_start(dy_sb, dy)
            gp_sb = consts.tile([P, ntiles], F32)
            nc.gpsimd.dma_start(gp_sb, gpc)
            zall = consts.tile([P, ntiles], F32)

            for g in range(ngroups):
                r0 = g * G
                hT = io3.tile([P, 2, G], MMDT, tag="hT")
                nc.sync.dma_start(hT, hidT_r[:, :, r0:r0 + G])

                # projT[d, r] = sum_h fc_wT[h, d] * hT[h, r]  (K=256)
                projT_ps = psA.tile([P, G], F32, tag="projT")
                nc.tensor.matmul(projT_ps, fcwT_sb[:, 0, :], hT[:, 0, :],
                                 start=True, stop=False)
                nc.tensor.matmul(projT_ps, fcwT_sb[:, 1, :], hT[:, 1, :],
                                 start=False, stop=True)
                projT_sb = work.tile([P, G], MMDT, tag="projT_sb")
                for q in range(4):
                    sl = slice(q * P, (q + 1) * P)
                    if q % 2 == 0:
                        nc.vector.tensor_scalar_add(projT_sb[:, sl],
                                                    projT_ps[:, sl], fcb_sb)
                    else:
                        nc.scalar.activation(
                            projT_sb[:, sl], projT_ps[:, sl],
                            mybir.ActivationFunctionType.Identity,
                            bias=fcb_sb)

                # raw[r, v] = sum_d projT[d, r] * metaT[d, v]  (K=128, N=512)
                raw_sb = io3.tile([P, 4, V], F32, tag="rawsb")
                for c in range(4):
                    raw_ps = psR.tile([P, V], F32, tag="raw")
                    nc.tensor.matmul(raw_ps, projT_sb[:, c * P:(c + 1) * P],
                                     metaT_sb, start=True, stop=True)
                    if c == 3:
                        nc.scalar.copy(raw_sb[:, c, :], raw_ps)
                    else:
                        nc.vector.tensor_copy(raw_sb[:, c, :], raw_ps)
                eng = nc.sync if g % 2 == 0 else nc.gpsimd
                eng.dma_start(
                    raw[r0:r0 + G, :].rearrange("(c p) v -> p c v", p=P),
                    raw_sb)

                # h1[p, r] = relu(l1w01 @ [div;yrs] + l1w2 @ proj + l1_b)
                h1_ps = psB.tile([PH, G], F32, tag="h1")
                nc.tensor.matmul(h1_ps, l1w01T_sb, dy_sb[:, r0:r0 + G],
                                 start=True, stop=False)
                nc.tensor.matmul(h1_ps, l1wT_sb, projT_sb,
                                 start=False, stop=True)
                h1_sb = work.tile([PH, G], SMDT, tag="h1sb")
                nc.scalar.activation(h1_sb, h1_ps,
                                     mybir.ActivationFunctionType.Relu,
                                     bias=l1b_sb)

                # z columns: z[r, 0] = sum_p h1[p, r] * l2_w[p]
                z_ps = psC.tile([P, 4], F32, tag="z")
                for c in range(4):
                    nc.tensor.matmul(z_ps[:, c:c + 1],
                                     h1_sb[:, c * P:(c + 1) * P], l2wT_sb,
                                     start=True, stop=True)
                nc.vector.tensor_copy(zall[:, g * 4:(g + 1) * 4], z_ps)

            # tail: ps = sigmoid(z + l2_b); w = ps*(1-2g) + g; logw = ln(w)
            onem2g = consts.tile([P, ntiles], F32)
            nc.vector.tensor_scalar(onem2g, gp_sb, -2.0, 1.0,
                                    mybir.AluOpType.mult, mybir.AluOpType.add)
            ps_all = consts.tile([P, ntiles], F32)
            nc.scalar.activation(ps_all, zall,
                                 mybir.ActivationFunctionType.Sigmoid,
                                 bias=l2b_sb)
            w_all = consts.tile([P, ntiles], F32)
            nc.vector.tensor_mul(w_all, ps_all, onem2g)
            nc.vector.tensor_add(w_all, w_all, gp_sb)
            logw_sb = consts.tile([P, ntiles], F32)
            nc.scalar.activation(logw_sb, w_all,
                                 mybir.ActivationFunctionType.Ln)
            nc.sync.dma_start(logwc, logw_sb)

    nc.compile()
    return nc


def _get_program(ncmax: int, mode: str):
    key = (ncmax, mode)
    if key not in _CACHE:
        _CACHE[key] = _build(ncmax, mode)
    return _CACHE[key]


def kernel(hidden_seq, seq_len, gp_sim_mask, div, yrs, meta_data,
           fc_w, fc_b, l1_w, l1_b, l2_w, l2_b, mode="bf16", trace=False):
    hidden_seq = np.asarray(hidden_seq, np.float32)
    sl = np.asarray(seq_len).astype(np.int64)
    gp = np.asarray(gp_sim_mask, np.float32).reshape(B * T)
    dv = np.asarray(div, np.float32).reshape(B * T)
    yr = np.asarray(yrs, np.float32).reshape(B * T)
    meta_data = np.asarray(meta_data, np.float32)
    fc_w = np.asarray(fc_w, np.float32)
    fc_b = np.asarray(fc_b, np.float32)
    l1_w = np.asarray(l1_w, np.float32)
    l1_b = np.asarray(l1_b, np.float32)
    l2_w = np.asarray(l2_w, np.float32)
    l2_b = np.asarray(l2_b, np.float32)

    # ragged row indices: for batch b, rows b*T + [0, seq_len[b]-1)
    lens = sl - 1
    N = int(lens.sum())
    idx = np.empty(N, np.int64)
    pos = 0
    for b in range(B):
        l = int(lens[b])
        idx[pos:pos + l] = b * T + np.arange(l)
        pos += l

    # contiguous split across cores
    bounds = [(N * c) // NCORES for c in range(NCORES + 1)]
    chunk = max(bounds[c + 1] - bounds[c] for c in range(NCORES))
    ncmax = max(G, ((chunk + G - 1) // G) * G)
    ntiles = ncmax // P

    import ml_dtypes
    mmnp = ml_dtypes.bfloat16 if mode == "bf16" else np.float32
    smnp = ml_dtypes.bfloat16 if mode == "bf16" else np.float32
    hid_flat = hidden_seq.reshape(B * T, H)
    shared = {
        "fcwT": np.ascontiguousarray(fc_w.T).astype(mmnp),
        "fcb": np.ascontiguousarray(fc_b.reshape(D, 1)),
        "metaT": np.ascontiguousarray(meta_data.T).astype(mmnp),
        "l1wT": np.ascontiguousarray(l1_w[:, 2:].T).astype(mmnp),
        "l1w01T": np.ascontiguousarray(l1_w[:, :2].T).astype(smnp),
        "l1b": np.ascontiguousarray(l1_b.reshape(PH, 1)),
        "l2wT": np.ascontiguousarray(l2_w.T).astype(smnp),
        "l2b": np.full((P, 1), np.float32(l2_b.reshape(1)[0])),
    }
    in_maps = []
    for c in range(NCORES):
        s, e = bounds[c], bounds[c + 1]
        n_c = e - s
        hidT_c = np.zeros((H, ncmax), mmnp)
        hidT_c[:, :n_c] = hid_flat[idx[s:e]].T.astype(mmnp)
        dy_c = np.zeros((2, ncmax), smnp)
        dy_c[0, :n_c] = dv[idx[s:e]].astype(smnp)
        dy_c[1, :n_c] = yr[idx[s:e]].astype(smnp)
        gp_c = np.zeros(ncmax, np.float32)
        gp_c[:n_c] = gp[idx[s:e]]
        gpc_c = np.ascontiguousarray(gp_c.reshape(ntiles, P).T)
        in_maps.append({"hidT": hidT_c, "dy": dy_c, "gpc": gpc_c, **shared})

    nc = _get_program(ncmax, mode)
    res = bass_utils.run_bass_kernel_spmd(nc, in_maps,
                                          core_ids=list(range(NCORES)),
                                          trace=trace)

    raw_out = np.empty((N, V), np.float32)
    logw_out = np.empty((N, 1), np.float32)
    for c in range(NCORES):
        s, e = bounds[c], bounds[c + 1]
        raw_out[s:e] = res.results[c]["raw"][:e - s]
        logw_flat = res.results[c]["logwc"].T.reshape(-1)
        logw_out[s:e, 0] = logw_flat[:e - s]
    if trace:
        kernel.last_results = res
    return raw_out, logw_out
